# revision 1
# baseline (speedup 1.0000x reference)
"""Trainium2 Bass kernel for DeformableConv2d (B,H,W,C=8,64,64,128; F=128; 3x3).

Strategy (data-parallel over batch, one batch element per NeuronCore):
  - Host: reshape inputs, precompute the (data-independent) base-grid constant
    and a channel-major weight layout. No data-dependent work on host.
  - Device, per core:
      * build x_pair in scratch DRAM: row q -> [x[q], x[q+64]] (zero padded),
        so one 512-element contiguous read at offset q*256 fetches the whole
        2x2 bilinear patch for integer corner q = y0*64 + x0.
      * index math on DVE: coords = grid + offset, clip, frac via mod,
        q = y0*64 + x0 (int32), 4 bilinear corner weights.
      * per pixel-tile (128 px): one indirect DMA gathers all 9 kernel
        points' patches [128, 9, 512]; DVE combines the 4 corners with
        per-partition scalar weights; PE transposes deform tiles into PSUM
        (channel-major); PE matmuls accumulate over the 9 kernel points into
        out^T; PE transposes back and results stream to DRAM.
"""

import os
from contextlib import ExitStack

import numpy as np

import concourse.bass as bass
import concourse.mybir as mybir
import concourse.tile as tile
from concourse import bacc
from concourse._compat import with_exitstack
from concourse.bass_utils import run_bass_kernel_spmd
from concourse.masks import make_identity

KH, KW, KN = 3, 3, 9
H = W_IMG = 64
C = 128
F = 128
P = 128
NPIX = H * W_IMG            # 4096 pixels per core
NT = NPIX // P              # 32 pixel tiles
NG = NT // 4                # 8 groups of 512 pixels
XROWS = NPIX + 1            # x rows incl. one zero pad row (host-padded)

f32 = mybir.dt.float32
bf16 = mybir.dt.bfloat16
i32 = mybir.dt.int32
ALU = mybir.AluOpType
ACT = mybir.ActivationFunctionType


def _grid_const():
    """reference._grid_offset in numpy, flattened to [4096, 18] then wrapped
    to the [128 partitions, 32*18] on-chip layout."""
    init = np.stack(np.meshgrid(np.arange(KH), np.arange(KW), indexing="ij"))
    init = init.reshape(-1, 2).astype(np.float32)
    ph, pw = (KH - 1) // 2, (KW - 1) // 2
    g = np.stack(
        np.meshgrid(np.arange(-ph, H - ph), np.arange(-pw, W_IMG - pw), indexing="ij"),
        axis=-1,
    ).astype(np.float32)
    full = (g[:, :, None, :] + init[None, None]).reshape(NPIX, 2 * KN)
    return np.ascontiguousarray(
        full.reshape(NT, P, 2 * KN).transpose(1, 0, 2).reshape(P, NT * 2 * KN)
    )


@with_exitstack
def _body(ctx: ExitStack, tc: "tile.TileContext", t_off, t_grid, t_w, t_b,
          t_out, t_xp, debug=False):
    nc = tc.nc
    off_ap = t_off.ap()
    grid_ap = t_grid.ap()
    w_ap = t_w.ap()
    b_ap = t_b.ap()
    out_ap = t_out.ap()
    xp_ap = t_xp.ap()

    const = ctx.enter_context(tc.tile_pool(name="const", bufs=1))
    idxp = ctx.enter_context(tc.tile_pool(name="idx", bufs=1))
    gpool = ctx.enter_context(tc.tile_pool(name="gath", bufs=8))
    dpool = ctx.enter_context(tc.tile_pool(name="deform", bufs=4))
    dTpool = ctx.enter_context(tc.tile_pool(name="dT", bufs=3))
    oTpool = ctx.enter_context(tc.tile_pool(name="oT", bufs=2))
    opool = ctx.enter_context(tc.tile_pool(name="o", bufs=4))
    ps_out = ctx.enter_context(tc.tile_pool(name="ps_out", bufs=2, space="PSUM"))
    ps_dT = ctx.enter_context(tc.tile_pool(name="ps_dT", bufs=2, space="PSUM"))
    ps_o = ctx.enter_context(tc.tile_pool(name="ps_o", bufs=2, space="PSUM"))

    # ---- constants ----
    ident = const.tile([P, P], f32)
    make_identity(nc, ident[:])
    ident16 = const.tile([P, P], bf16)
    nc.vector.tensor_copy(ident16[:], ident[:])
    w_sb = const.tile([P, KN, F], bf16)
    nc.sync.dma_start(w_sb[:], w_ap)  # [C, KN, F] bf16, c on partitions
    b_sb = const.tile([P, 1], f32)
    nc.sync.dma_start(b_sb[:], b_ap[:, None])

    xp3 = xp_ap.rearrange("q (j c) -> q j c", j=2)

    # ---- load offsets + grid ----
    offs = idxp.tile([P, NT, 2 * KN], f32)
    nc.sync.dma_start(offs[:], off_ap.rearrange("(t p) k -> p t k", p=P))
    grid = idxp.tile([P, NT, 2 * KN], f32)
    nc.sync.dma_start(grid[:], grid_ap.rearrange("p (t k) -> p t k", k=2 * KN))

    # ---- index math (all tiles at once) ----
    co = idxp.tile([P, NT, 2 * KN], f32)
    nc.vector.tensor_add(co[:], offs[:], grid[:])
    nc.vector.tensor_scalar(co[:], co[:], 0.0, float(H - 1), ALU.max, ALU.min)
    # floor via int round-trip; works for round-to-nearest (HW) and trunc (sim):
    # r = float(int(y)); floor = r - (r > y)
    ci = idxp.tile([P, NT, 2 * KN], i32)
    nc.vector.tensor_copy(ci[:], co[:])
    cf = idxp.tile([P, NT, 2 * KN], f32)
    nc.vector.tensor_copy(cf[:], ci[:])
    gt = idxp.tile([P, NT, 2 * KN], f32)
    nc.vector.tensor_tensor(gt[:], cf[:], co[:], ALU.is_gt)
    c0 = idxp.tile([P, NT, 2 * KN], f32)
    nc.vector.tensor_sub(c0[:], cf[:], gt[:])
    fr = idxp.tile([P, NT, 2 * KN], f32)
    nc.vector.tensor_sub(fr[:], co[:], c0[:])
    un = idxp.tile([P, NT, 2 * KN], f32)
    nc.vector.tensor_scalar(un[:], fr[:], -1.0, 1.0, ALU.mult, ALU.add)

    c0v = c0[:].rearrange("p t (n two) -> p t n two", two=2)
    frv = fr[:].rearrange("p t (n two) -> p t n two", two=2)
    unv = un[:].rearrange("p t (n two) -> p t n two", two=2)

    qf = idxp.tile([P, NT, KN], f32)
    nc.vector.scalar_tensor_tensor(
        qf[:], c0v[:, :, :, 0], 64.0, c0v[:, :, :, 1], ALU.mult, ALU.add
    )
    # kn-major int index tile; [128, 1] slices are contiguous for the DMA
    qi = idxp.tile([P, KN, NT], i32)
    nc.vector.tensor_copy(qi[:].rearrange("p n t -> p t n"), qf[:])

    # corner weights [00, 10, 01, 11]; rows ~ y (index 0), cols ~ x (index 1)
    w4 = idxp.tile([P, NT, KN, 4], f32)
    nc.vector.tensor_tensor(w4[:, :, :, 0], unv[:, :, :, 0], unv[:, :, :, 1], ALU.mult)
    nc.vector.tensor_tensor(w4[:, :, :, 1], frv[:, :, :, 0], unv[:, :, :, 1], ALU.mult)
    nc.vector.tensor_tensor(w4[:, :, :, 2], unv[:, :, :, 0], frv[:, :, :, 1], ALU.mult)
    nc.vector.tensor_tensor(w4[:, :, :, 3], frv[:, :, :, 0], frv[:, :, :, 1], ALU.mult)

    if debug:
        d_q = nc.dram_tensor("dbg_q", [P, KN * NG * 8], i32, kind="ExternalOutput")
        d_w4 = nc.dram_tensor("dbg_w4", [P, NT * KN * 4], f32, kind="ExternalOutput")
        d_g = nc.dram_tensor("dbg_g", [P, 4 * C], f32, kind="ExternalOutput")
        d_dt = nc.dram_tensor("dbg_dt", [P, 512], f32, kind="ExternalOutput")
        d_ot = nc.dram_tensor("dbg_ot", [P, 512], f32, kind="ExternalOutput")
        nc.sync.dma_start(d_q.ap().rearrange("p (n g j) -> p n g j", n=KN, g=NG), qall[:])
        nc.sync.dma_start(
            d_w4.ap().rearrange("p (t n j) -> p t n j", t=NT, n=KN), w4[:]
        )

    # ---- main loop ----
    for g in range(NG):
        ops = ps_out.tile([P, 512], f32)  # out^T accumulator [f, 512 px]
        for kn in range(KN):
            dps = ps_dT.tile([P, 512], bf16)  # deform^T [c, 512 px]
            for t4 in range(4):
                t = g * 4 + t4
                # one gather per (tile, kn): pair rows q, q+1 of x_pair =
                # corners [00 | 10 | 01 | 11], 2KB per descriptor
                G = gpool.tile([P, 4 * C], bf16)
                nc.gpsimd.indirect_dma_start(
                    out=G[:], out_offset=None, in_=xp3[:, :, :],
                    in_offset=bass.IndirectOffsetOnAxis(
                        ap=qi[:, kn, t : t + 1], axis=0),
                )
                d = dpool.tile([P, C], bf16)
                nc.vector.tensor_scalar_mul(d[:], G[:, 0:C], w4[:, t, kn, 0:1])
                for blk in (1, 2, 3):
                    nc.vector.scalar_tensor_tensor(
                        d[:],
                        G[:, blk * C : (blk + 1) * C],
                        w4[:, t, kn, blk : blk + 1],
                        d[:],
                        ALU.mult,
                        ALU.add,
                    )
                nc.tensor.transpose(dps[:, t4 * P : (t4 + 1) * P], d[:], ident16[:])
            dT = dTpool.tile([P, 512], bf16)
            nc.scalar.copy(dT[:], dps[:])
            nc.tensor.matmul(
                ops[:], lhsT=w_sb[:, kn, :], rhs=dT[:],
                start=(kn == 0), stop=(kn == KN - 1),
            )
        oT = oTpool.tile([P, 512], f32)
        nc.scalar.activation(oT[:], ops[:], ACT.Identity, bias=b_sb[:, 0:1], scale=1.0)
        if debug and g == 0:
            nc.sync.dma_start(d_ot.ap(), oT[:])
        for t4 in range(4):
            o_ps = ps_o.tile([P, P], f32)
            nc.tensor.transpose(o_ps[:], oT[:, t4 * P : (t4 + 1) * P], ident[:])
            o_sb = opool.tile([P, P], f32)
            nc.scalar.copy(o_sb[:], o_ps[:])
            pix0 = (g * 4 + t4) * P
            nc.sync.dma_start(out_ap[pix0 : pix0 + P, :], o_sb[:])


def build_nc(debug=False):
    nc = bacc.Bacc(
        "TRN2",
        target_bir_lowering=False,
        debug=False,
        enable_asserts=False,
        num_devices=8,
    )
    t_off = nc.dram_tensor("off", [NPIX, 2 * KN], f32, kind="ExternalInput")
    t_grid = nc.dram_tensor("grid", [P, NT * 2 * KN], f32, kind="ExternalInput")
    t_w = nc.dram_tensor("w", [C, KN, F], bf16, kind="ExternalInput")
    t_b = nc.dram_tensor("b", [F], f32, kind="ExternalInput")
    t_out = nc.dram_tensor("out", [NPIX, F], f32, kind="ExternalOutput")
    t_xp = nc.dram_tensor("xpair", [NPIX + 2, 2 * C], bf16, kind="ExternalInput")
    with tile.TileContext(nc) as tc:
        _body(tc, t_off, t_grid, t_w, t_b, t_out, t_xp, debug=debug)
    nc.compile()
    return nc


def make_in_maps(x, offset, W, b):
    B = x.shape[0]
    grid_host = _grid_const()
    import ml_dtypes
    w_host = np.ascontiguousarray(
        np.asarray(W, np.float32).transpose(1, 0, 2).astype(ml_dtypes.bfloat16))
    b_host = np.ascontiguousarray(np.asarray(b, np.float32))
    in_maps = []
    for i in range(B):
        xi = np.asarray(x[i], np.float32).reshape(NPIX, C).astype(ml_dtypes.bfloat16)
        xp = np.zeros((NPIX + 2, 2, C), ml_dtypes.bfloat16)
        xp[:NPIX, 0, :] = xi
        xp[: NPIX - 64, 1, :] = xi[64:]
        in_maps.append(
            {
                "xpair": np.ascontiguousarray(xp.reshape(NPIX + 2, 2 * C)),
                "off": np.ascontiguousarray(
                    np.asarray(offset[i], np.float32).reshape(NPIX, 2 * KN)
                ),
                "grid": grid_host,
                "w": w_host,
                "b": b_host,
            }
        )
    return in_maps


_RESULTS_CACHE = {}


def kernel(x, offset, W, b, _trace=False):
    x = np.asarray(x)
    B = x.shape[0]
    assert x.shape == (B, H, W_IMG, C), x.shape
    nc = build_nc()
    in_maps = make_in_maps(x, offset, W, b)
    res = run_bass_kernel_spmd(nc, in_maps, core_ids=list(range(B)), trace=_trace)
    _RESULTS_CACHE["last"] = res
    out = np.stack(
        [res.results[i]["out"].reshape(H, W_IMG, F) for i in range(B)]
    ).astype(np.float32)
    return out



# revision 6
# speedup vs baseline: 1.3381x; 1.3381x over previous
"""Trainium2 Bass kernel for DeformableConv2d (B,H,W,C=8,64,64,128; F=128; 3x3).

Data-parallel over batch: one batch element per NeuronCore (8 cores).

v2 design notes (vs v1 which used 288 per-tile indirect DMAs):
  - Gathers batched via gpsimd dma_gather: one instruction per (kernel point,
    block of 8 pixel tiles) = 36 gathers x 1024 descriptors of 1KB. SWDGE
    fixed cost (~1us/instruction) amortized 8x.
  - xquad DRAM layout: row q = [x[q], x[q+64], x[q+1], x[q+65]] so one 1KB
    contiguous read fetches all 4 bilinear corners for integer corner
    q = y0*64 + x0.
  - floor via round(y - 0.5): grid constant is pre-shifted by -0.5 on host and
    clipped to [-0.499, 62.499]; round-to-nearest of that equals floor(y)
    (or floor(y)-1 with frac weight exactly 1 at integer y - same lerp value).
    No floor/ceil correction ops, and q stays in [0, 4030] so no padding rows.
  - dma_gather wants indices as int16 in a [128, n/16] tile: index i at
    [i%16, i//16], replicated across the 8 gpsimd-core partition groups. A
    second tiny coordinate pipeline on partitions 0..15 computes q in exactly
    that layout from a host-permuted copy of the offsets; 8 small SBUF->SBUF
    DMAs replicate it.
  - Bilinear combine on DVE in pixel-major layout, batched per (kn, block):
    one [128,8,4,128] mult against broadcast corner weights + 3 adds.
  - PE transposes combined tiles to channel-major, matmuls accumulate over
    the 9 kernel points in PSUM, bias via activation, PE transposes back.
"""

import os
from contextlib import ExitStack

import numpy as np

import concourse.bass as bass
import concourse.mybir as mybir
import concourse.tile as tile
from concourse import bacc
from concourse._compat import with_exitstack
from concourse.bass_utils import run_bass_kernel_spmd
from concourse.masks import make_identity

KH, KW, KN = 3, 3, 9
H = W_IMG = 64
C = 128
F = 128
P = 128
NPIX = H * W_IMG            # 4096 pixels per core
NT = NPIX // P              # 32 pixel tiles
NB = 4                      # blocks of 8 tiles
TB = NT // NB               # 8 tiles per block
NIDX = TB * P               # 1024 gathered pixels per dma_gather
ES = 4 * C                  # 512 elems (1KB bf16) per gather descriptor

f32 = mybir.dt.float32
bf16 = mybir.dt.bfloat16
i32 = mybir.dt.int32
i16 = mybir.dt.int16
ALU = mybir.AluOpType
ACT = mybir.ActivationFunctionType

CLIP_LO = -0.499
CLIP_HI = 62.499


def _grid_full():
    """reference._grid_offset flattened to [4096, 18] (y,x interleaved)."""
    init = np.stack(np.meshgrid(np.arange(KH), np.arange(KW), indexing="ij"))
    init = init.reshape(-1, 2).astype(np.float32)
    ph, pw = (KH - 1) // 2, (KW - 1) // 2
    g = np.stack(
        np.meshgrid(np.arange(-ph, H - ph), np.arange(-pw, W_IMG - pw), indexing="ij"),
        axis=-1,
    ).astype(np.float32)
    return (g[:, :, None, :] + init[None, None]).reshape(NPIX, 2 * KN)


@with_exitstack
def _body(ctx: ExitStack, tc: "tile.TileContext", t_off, t_grid, t_off2,
          t_grid2, t_w, t_b, t_out, t_xq):
    nc = tc.nc
    off_ap = t_off.ap()
    grid_ap = t_grid.ap()
    off2_ap = t_off2.ap()
    grid2_ap = t_grid2.ap()
    w_ap = t_w.ap()
    b_ap = t_b.ap()
    out_ap = t_out.ap()
    xq_ap = t_xq.ap()

    const = ctx.enter_context(tc.tile_pool(name="const", bufs=1))
    idxp = ctx.enter_context(tc.tile_pool(name="idx", bufs=1))
    gpool = ctx.enter_context(tc.tile_pool(name="gath", bufs=4))
    mpool = ctx.enter_context(tc.tile_pool(name="m", bufs=2))
    abpool = ctx.enter_context(tc.tile_pool(name="ab", bufs=4))
    dpool = ctx.enter_context(tc.tile_pool(name="d", bufs=3))
    dTpool = ctx.enter_context(tc.tile_pool(name="dT", bufs=3))
    oTpool = ctx.enter_context(tc.tile_pool(name="oT", bufs=4))
    opool = ctx.enter_context(tc.tile_pool(name="o", bufs=4))
    ps_dT = ctx.enter_context(tc.tile_pool(name="ps_dT", bufs=2, space="PSUM"))
    ps_out = ctx.enter_context(tc.tile_pool(name="ps_out", bufs=2, space="PSUM"))
    ps_o = ctx.enter_context(tc.tile_pool(name="ps_o", bufs=2, space="PSUM"))

    # ---- constants ----
    ident = const.tile([P, P], f32)
    make_identity(nc, ident[:])
    ident16 = const.tile([P, P], bf16)
    nc.vector.tensor_copy(ident16[:], ident[:])
    w_sb = const.tile([P, KN, F], bf16)
    nc.sync.dma_start(w_sb[:], w_ap)  # [C, KN, F] bf16
    b_sb = const.tile([P, 1], f32)
    nc.sync.dma_start(b_sb[:], b_ap[:, None])

    # ---- corner-weight pipeline (pixel px = tau*128 + p at [p, tau]) ----
    offs = idxp.tile([P, NT, 2 * KN], f32)
    nc.sync.dma_start(offs[:], off_ap.rearrange("(t p) k -> p t k", p=P))
    grid = idxp.tile([P, NT, 2 * KN], f32)
    nc.sync.dma_start(grid[:], grid_ap.rearrange("p (t k) -> p t k", k=2 * KN))

    co = idxp.tile([P, NT, 2 * KN], f32)
    nc.vector.tensor_add(co[:], offs[:], grid[:])
    nc.vector.tensor_scalar(co[:], co[:], CLIP_LO, CLIP_HI, ALU.max, ALU.min)
    ci = idxp.tile([P, NT, 2 * KN], i32)
    nc.vector.tensor_copy(ci[:], co[:])            # round-to-nearest
    cf = idxp.tile([P, NT, 2 * KN], f32)
    nc.vector.tensor_copy(cf[:], ci[:])
    fr = idxp.tile([P, NT, 2 * KN], f32)
    nc.vector.tensor_sub(fr[:], co[:], cf[:])      # in [-0.5, 0.5]
    fp = idxp.tile([P, NT, 2 * KN], f32)           # frac = fr + 0.5
    nc.vector.tensor_scalar_add(fp[:], fr[:], 0.5)
    un = idxp.tile([P, NT, 2 * KN], f32)           # 1 - frac = 0.5 - fr
    nc.vector.tensor_scalar(un[:], fr[:], -1.0, 0.5, ALU.mult, ALU.add)

    fpv = fp[:].rearrange("p t (n two) -> p t n two", two=2)
    unv = un[:].rearrange("p t (n two) -> p t n two", two=2)

    # corner weights, order [00, 10, 01, 11] matching xquad layout
    w4 = idxp.tile([P, NT, KN, 4], bf16)
    nc.vector.tensor_tensor(w4[:, :, :, 0], unv[:, :, :, 0], unv[:, :, :, 1], ALU.mult)
    nc.vector.tensor_tensor(w4[:, :, :, 1], fpv[:, :, :, 0], unv[:, :, :, 1], ALU.mult)
    nc.vector.tensor_tensor(w4[:, :, :, 2], unv[:, :, :, 0], fpv[:, :, :, 1], ALU.mult)
    nc.vector.tensor_tensor(w4[:, :, :, 3], fpv[:, :, :, 0], fpv[:, :, :, 1], ALU.mult)

    # ---- gather-index pipeline on partitions 0..15 ----
    # layout [a, kn, coord, tau, phi]: pixel (p = phi*16+a, tau)
    off2 = idxp.tile([16, KN, 2, NT, 8], f32)
    nc.sync.dma_start(off2[:], off2_ap.rearrange("a (k c t h) -> a k c t h",
                                                 k=KN, c=2, t=NT))
    grid2 = idxp.tile([16, KN, 2, NT, 8], f32)
    nc.sync.dma_start(grid2[:], grid2_ap.rearrange("a (k c t h) -> a k c t h",
                                                   k=KN, c=2, t=NT))
    co2 = idxp.tile([16, KN, 2, NT, 8], f32)
    nc.vector.tensor_add(co2[:], off2[:], grid2[:])
    nc.vector.tensor_scalar(co2[:], co2[:], CLIP_LO, CLIP_HI, ALU.max, ALU.min)
    ci2 = idxp.tile([16, KN, 2, NT, 8], i32)
    nc.vector.tensor_copy(ci2[:], co2[:])
    q32 = idxp.tile([16, KN, NT, 8], i32)
    nc.vector.tensor_scalar(q32[:], ci2[:, :, 0, :, :], 6, None,
                            ALU.arith_shift_left)
    nc.vector.tensor_tensor(q32[:], q32[:], ci2[:, :, 1, :, :], ALU.add)
    qw16 = idxp.tile([16, KN, NT, 8], i16)
    nc.vector.tensor_copy(qw16[:], q32[:])
    # replicate to all 8 gpsimd-core partition groups
    qw = idxp.tile([P, KN, NT, 8], i16)
    for g in range(8):
        nc.sync.dma_start(qw[g * 16:(g + 1) * 16, :, :, :], qw16[:])

    # ---- main loop ----
    for b in range(NB):
        ops0 = ps_out.tile([P, 512], f32)
        ops1 = ps_out.tile([P, 512], f32)
        for kn in range(KN):
            G = gpool.tile([P, TB, ES], bf16)
            nc.gpsimd.dma_gather(
                G[:], xq_ap, qw[:, kn, b * TB:(b + 1) * TB, :], NIDX, NIDX, ES,
            )
            m = mpool.tile([P, TB, 4, C], bf16)
            nc.vector.tensor_tensor(
                m[:],
                G[:].rearrange("p t (j c) -> p t j c", j=4),
                w4[:, b * TB:(b + 1) * TB, kn, :].unsqueeze(3)
                    .broadcast_to([P, TB, 4, C]),
                ALU.mult,
            )
            aa = abpool.tile([P, TB, C], bf16)
            nc.vector.tensor_tensor(aa[:], m[:, :, 0, :], m[:, :, 1, :], ALU.add)
            bb = abpool.tile([P, TB, C], bf16)
            nc.vector.tensor_tensor(bb[:], m[:, :, 2, :], m[:, :, 3, :], ALU.add)
            dd = dpool.tile([P, TB, C], bf16)
            nc.vector.tensor_tensor(dd[:], aa[:], bb[:], ALU.add)

            dps = ps_dT.tile([P, TB, P], bf16)
            for tl in range(TB):
                nc.tensor.transpose(dps[:, tl, :], dd[:, tl, :], ident16[:])
            dT = dTpool.tile([P, TB, P], bf16)
            nc.scalar.copy(dT[:], dps[:])
            nc.tensor.matmul(
                ops0[:], lhsT=w_sb[:, kn, :],
                rhs=dT[:, 0:4, :].rearrange("p t c -> p (t c)"),
                start=(kn == 0), stop=(kn == KN - 1),
            )
            nc.tensor.matmul(
                ops1[:], lhsT=w_sb[:, kn, :],
                rhs=dT[:, 4:8, :].rearrange("p t c -> p (t c)"),
                start=(kn == 0), stop=(kn == KN - 1),
            )
        for half, ops in ((0, ops0), (1, ops1)):
            oT = oTpool.tile([P, 512], f32)
            nc.scalar.activation(oT[:], ops[:], ACT.Identity,
                                 bias=b_sb[:, 0:1], scale=1.0)
            for t4 in range(4):
                o_ps = ps_o.tile([P, P], f32)
                nc.tensor.transpose(o_ps[:], oT[:, t4 * P:(t4 + 1) * P], ident[:])
                o_sb = opool.tile([P, P], f32)
                nc.scalar.copy(o_sb[:], o_ps[:])
                pix0 = (b * TB + half * 4 + t4) * P
                nc.sync.dma_start(out_ap[pix0:pix0 + P, :], o_sb[:])


def build_nc():
    nc = bacc.Bacc(
        "TRN2",
        target_bir_lowering=False,
        debug=False,
        enable_asserts=False,
        num_devices=8,
    )
    t_off = nc.dram_tensor("off", [NPIX, 2 * KN], f32, kind="ExternalInput")
    t_grid = nc.dram_tensor("grid", [P, NT * 2 * KN], f32, kind="ExternalInput")
    t_off2 = nc.dram_tensor("off2", [16, KN * 2 * NT * 8], f32, kind="ExternalInput")
    t_grid2 = nc.dram_tensor("grid2", [16, KN * 2 * NT * 8], f32, kind="ExternalInput")
    t_w = nc.dram_tensor("w", [C, KN, F], bf16, kind="ExternalInput")
    t_b = nc.dram_tensor("b", [F], f32, kind="ExternalInput")
    t_out = nc.dram_tensor("out", [NPIX, F], f32, kind="ExternalOutput")
    t_xq = nc.dram_tensor("xquad", [NPIX, ES], bf16, kind="ExternalInput")
    with tile.TileContext(nc) as tc:
        _body(tc, t_off, t_grid, t_off2, t_grid2, t_w, t_b, t_out, t_xq)
    nc.compile()
    return nc


def make_in_maps(x, offset, W, b):
    import ml_dtypes

    B = x.shape[0]
    grid_full = _grid_full()                       # [4096, 18]
    grid_host = np.ascontiguousarray(
        (grid_full.reshape(NT, P, 2 * KN).transpose(1, 0, 2) - 0.5)
        .reshape(P, NT * 2 * KN)
    ).astype(np.float32)
    # [a, kn, coord, tau, phi] <- full[tau*128 + phi*16 + a, 2kn+coord]
    g5 = grid_full.reshape(NT, 8, 16, KN, 2).transpose(2, 3, 4, 0, 1) - 0.5
    grid2_host = np.ascontiguousarray(g5.reshape(16, KN * 2 * NT * 8)).astype(np.float32)

    w_host = np.ascontiguousarray(
        np.asarray(W, np.float32).transpose(1, 0, 2).astype(ml_dtypes.bfloat16))
    b_host = np.ascontiguousarray(np.asarray(b, np.float32))

    in_maps = []
    for i in range(B):
        xi = np.asarray(x[i], np.float32).reshape(NPIX, C).astype(ml_dtypes.bfloat16)
        xq = np.zeros((NPIX, 4, C), ml_dtypes.bfloat16)
        xq[:, 0, :] = xi
        xq[:-64, 1, :] = xi[64:]
        xq[:-1, 2, :] = xi[1:]
        xq[:-65, 3, :] = xi[65:]
        off_i = np.asarray(offset[i], np.float32).reshape(NPIX, 2 * KN)
        off2_i = np.ascontiguousarray(
            off_i.reshape(NT, 8, 16, KN, 2).transpose(2, 3, 4, 0, 1)
            .reshape(16, KN * 2 * NT * 8)
        )
        in_maps.append(
            {
                "xquad": np.ascontiguousarray(xq.reshape(NPIX, ES)),
                "off": np.ascontiguousarray(off_i),
                "off2": off2_i,
                "grid": grid_host,
                "grid2": grid2_host,
                "w": w_host,
                "b": b_host,
            }
        )
    return in_maps


_RESULTS_CACHE = {}


def kernel(x, offset, W, b, _trace=False):
    x = np.asarray(x)
    B = x.shape[0]
    assert x.shape == (B, H, W_IMG, C), x.shape
    nc = build_nc()
    in_maps = make_in_maps(x, offset, W, b)
    res = run_bass_kernel_spmd(nc, in_maps, core_ids=list(range(B)), trace=_trace)
    _RESULTS_CACHE["last"] = res
    out = np.stack(
        [res.results[i]["out"].reshape(H, W_IMG, F) for i in range(B)]
    ).astype(np.float32)
    return out
